# revision 24
# baseline (speedup 1.0000x reference)
"""KascadeReuseAttention Trainium2 kernel (v2).

Sharding: 16 heads / 8 cores -> 2 heads per core (head/tensor parallel).
Wq/Wk/Wv column-sharded by head, Wo row-sharded; host sums the 8 partial
outputs (the row-parallel all-reduce).

Single SPMD program for all cores: per-core anchor selection enters only as
DATA. Per (head, query-tile t) we compute block attention against ALL past
key tiles v<=t; tile multiplicities m[h,t,v] (count of v among anchors+local,
0 if unselected) are folded into the LOGITS as additive biases
B = sqrt(D)*ln(m) (-4e9 when m=0), accumulated into the logits PSUM by a
tiny K=1 matmul reading a host-precomputed bias row. exp() then yields
m*exp(s*qk) (or 0) with no per-pair vector work. The causal tri-mask for the
diagonal tile is likewise one additive matmul (identity x trineg).

The denominator rides in the PV matmul: V tiles carry an appended
ones-column, so PV produces [q, D+1] with the last column = sum of weights.
Normalization is then a per-partition reciprocal + scalar multiply, and the
[q, d] -> [d, q] transpose for the output projection is a DMA transpose.
"""

import math
import sys

import numpy as np

for _p in ("/opt/trn_rl_repo",):
    if _p not in sys.path:
        sys.path.insert(0, _p)

import ml_dtypes  # noqa: E402
import concourse.bass as bass  # noqa: E402
import concourse.mybir as mybir  # noqa: E402
import concourse.tile as tile  # noqa: E402
from concourse.bass_utils import run_bass_kernel_spmd  # noqa: E402
from concourse.vector_clock import ScopedClock  # noqa: E402

BF16 = mybir.dt.bfloat16
F32 = mybir.dt.float32
NPBF16 = ml_dtypes.bfloat16

B, S, E, H, D, K = 1, 4096, 2048, 16, 128, 8
TILE = 128
T = S // TILE          # 32 query/key tiles
NCORES = 8
HPC = H // NCORES      # heads per core = 2
CHUNK = 512            # s-chunk for projections
NCHUNK = S // CHUNK
EK = E // TILE         # 16 contraction tiles
SM_SCALE = 1.0 / math.sqrt(D)
GRP = 4                # logits tiles per psum bank
NEGB = -4e9            # additive bias for m=0 pairs (exp -> 0)
NEGT = -1e9            # additive causal mask value

_PATCHED = False


def _patch_tile_drain():
    """This container's walrus caps per-instruction sync waits; the Tile
    kernel-tail drain carries one wait per live semaphore. Split them onto
    preceding SP nops."""
    global _PATCHED
    if _PATCHED:
        return
    _PATCHED = True

    def _drain_and_barrier(self, tick_clock, wait_clock):
        nc = self.nc
        nops = []
        nsems = len(self.sems.allocated()) if self.sems is not None else 0
        for _ in range(nsems):
            nops.append(nc.sync.nop())
        drain_inst = nc.sync.drain()
        wait_clock.add_sem_waits(
            drain_inst.ins, ScopedClock({None: tick_clock.global_clock})
        )
        si = drain_inst.ins.sync_info
        waits = list(si.on_wait or [])
        if len(waits) > 1:
            si.on_wait = waits[:1]
            for i, w in enumerate(waits[1:]):
                ni = nops[i].ins if hasattr(nops[i], "ins") else nops[i]
                nsi = ni.sync_info
                if nsi is None:
                    ni.sync_info = mybir.SyncInfo(on_wait=[w], on_update=[])
                else:
                    nsi.on_wait = [w]
        nc.all_engine_barrier()
        assert self.sems is not None
        popped = nc._tile_sem_poison_stack.pop()
        assert popped is self._sem_poison
        nc.clear_and_free_semaphores(list(self.sems.allocated().values()))
        nc.all_engine_barrier()
        _split_multi_waits(nc)

    tile.TileContext._drain_and_barrier = _drain_and_barrier


def _split_multi_waits(nc):
    """Walrus here encodes at most one sync-wait per instruction; move the
    extras onto preceding same-engine no-ops."""
    ctr = [0]
    for f in nc.m.functions:
        for bb in f.blocks:
            insts = list(bb.instructions)
            if not any(
                i.sync_info and i.sync_info.on_wait
                and len(i.sync_info.on_wait) > 1
                for i in insts
            ):
                continue
            newl = []
            for inst in insts:
                si = inst.sync_info
                if si and si.on_wait and len(si.on_wait) > 1:
                    waits = list(si.on_wait)
                    for w in waits[:-1]:
                        ctr[0] += 1
                        nop = mybir.InstNoOp(
                            name=f"WSPL-{ctr[0]}", ins=[], outs=[])
                        nop.engine = inst.engine
                        nop.sync_info = mybir.SyncInfo(
                            on_wait=[w], on_update=[])
                        newl.append(nop)
                    si.on_wait = waits[-1:]
                newl.append(inst)
            bb.instructions = newl


def build_bass():
    """Uniform per-core program. Inputs (per core, bf16 unless noted):
    xT [E, S], wqk [E, 4*128] (q_h0,q_h1,k_h0,k_h1), wv [E, 256],
    wo [256, E], cosT/sinT [128, S], rotT [128,128] (R^T for rotate_half),
    ident [128,128], trineg [128,128] (additive causal mask, -1e9 below
    diag), mwx [128, T*128] (bias rows: partition h*32+t holds, at col
    j*128+i, the value sqrt(D)*ln(m[h,t,j]) or -4e9).
    Output: outT [E, S] bf16 (partial contribution of this core's heads).
    """
    nc = bass.Bass()
    xT = nc.dram_tensor("xT", [E, S], BF16, kind="ExternalInput")
    wqk = nc.dram_tensor("wqk", [E, 4 * TILE], BF16, kind="ExternalInput")
    wv = nc.dram_tensor("wv", [E, 2 * TILE], BF16, kind="ExternalInput")
    wo = nc.dram_tensor("wo", [2 * TILE, E], BF16, kind="ExternalInput")
    cosT = nc.dram_tensor("cosT", [TILE, S], BF16, kind="ExternalInput")
    sinT = nc.dram_tensor("sinT", [TILE, S], BF16, kind="ExternalInput")
    rotT = nc.dram_tensor("rotT", [TILE, TILE], BF16, kind="ExternalInput")
    identD = nc.dram_tensor("identD", [TILE, TILE], BF16,
                            kind="ExternalInput")
    trinegD = nc.dram_tensor("trinegD", [TILE, TILE], BF16,
                             kind="ExternalInput")
    mwxD = nc.dram_tensor("mwxD", [2 * T, T * TILE], BF16,
                          kind="ExternalInput")
    ohD = nc.dram_tensor("ohD", [2 * T, T * TILE], BF16,
                         kind="ExternalInput")
    outT = nc.dram_tensor("outT", [E, S], BF16, kind="ExternalOutput")

    with tile.TileContext(nc) as tc:
        with tc.tile_pool(name="const", bufs=1) as cpool:
            sb_wqk = cpool.tile([TILE, EK, 4 * TILE], BF16)
            sb_wv = cpool.tile([TILE, EK, 2 * TILE], BF16)
            sb_wo = cpool.tile([TILE, 2, E], BF16)
            sb_cos = cpool.tile([TILE, S], BF16)
            sb_sin = cpool.tile([TILE, S], BF16)
            sb_rot = cpool.tile([TILE, TILE], BF16)
            sb_id = cpool.tile([TILE, TILE], BF16)
            sb_tn = cpool.tile([TILE, TILE], BF16)
            sb_mwx = cpool.tile([2 * T, T * TILE], BF16)
            sb_oh = cpool.tile([2 * T, T * TILE], BF16)
            # persistent per-head tensors: qT/kT [d, S]; v [kv, 132] per
            # tile with col 128 = 1.0 (denominator column); attnT [d, S]
            sb_q = cpool.tile([TILE, HPC, S], BF16, tag="q")
            sb_k = cpool.tile([TILE, HPC, S], BF16, tag="k")
            sb_vg = cpool.tile([TILE, HPC, T, 132], BF16, tag="vg")
            sb_attn = cpool.tile([TILE, HPC, S], BF16, tag="attn")

            nc.sync.dma_start(out=sb_wqk[:],
                              in_=wqk.rearrange("(a p) b -> p a b", p=TILE))
            nc.sync.dma_start(out=sb_wv[:],
                              in_=wv.rearrange("(a p) b -> p a b", p=TILE))
            nc.sync.dma_start(out=sb_wo[:],
                              in_=wo.rearrange("(a p) b -> p a b", p=TILE))
            nc.sync.dma_start(out=sb_cos[:], in_=cosT[:])
            nc.sync.dma_start(out=sb_sin[:], in_=sinT[:])
            nc.sync.dma_start(out=sb_rot[:], in_=rotT[:])
            nc.sync.dma_start(out=sb_id[:], in_=identD[:])
            nc.sync.dma_start(out=sb_tn[:], in_=trinegD[:])
            nc.sync.dma_start(out=sb_mwx[:], in_=mwxD[:])
            nc.sync.dma_start(out=sb_oh[:], in_=ohD[:])
            nc.vector.memset(sb_vg[:, :, :, 128:129], 1.0)

            with (
                tc.tile_pool(name="xin", bufs=2) as xpool,
                tc.tile_pool(name="rawp", bufs=3) as rawp,
                tc.tile_pool(name="t1p", bufs=3) as t1p,
                tc.tile_pool(name="t2p", bufs=3) as t2p,
                tc.tile_pool(name="wtp", bufs=3) as wtp,
                tc.tile_pool(name="nrm", bufs=3) as nrm,
                tc.tile_pool(name="obp", bufs=2) as obp,
                tc.tile_pool(name="pp", bufs=3, space="PSUM") as pp,
                tc.tile_pool(name="lg", bufs=2, space="PSUM") as lgp_pool,
                tc.tile_pool(name="oo", bufs=2, space="PSUM") as oo,
                tc.tile_pool(name="trp", bufs=1, space="PSUM") as trp,
            ):
                for ci in range(NCHUNK):
                    s0 = ci * CHUNK
                    xt = xpool.tile([TILE, EK, CHUNK], BF16, tag="xt")
                    nc.sync.dma_start(
                        out=xt[:],
                        in_=xT[:, s0:s0 + CHUNK].rearrange(
                            "(a p) b -> p a b", p=TILE),
                    )
                    # ---- projections + RoPE for this chunk ----
                    # qT/kT M-tiles: 0=q_h0 1=q_h1 2=k_h0 3=k_h1
                    for m in range(4):
                        ps = pp.tile([TILE, CHUNK], F32, tag="ps")
                        for e in range(EK):
                            nc.tensor.matmul(
                                ps[:], sb_wqk[:, e, m * TILE:(m + 1) * TILE],
                                xt[:, e, :], start=(e == 0), stop=(e == EK - 1))
                        raw = rawp.tile([TILE, CHUNK], BF16, tag="raw")
                        nc.scalar.copy(out=raw[:], in_=ps[:])
                        rot = pp.tile([TILE, CHUNK], F32, tag="ps")
                        nc.tensor.matmul(rot[:], sb_rot[:], raw[:],
                                         start=True, stop=True)
                        t1 = t1p.tile([TILE, CHUNK], BF16, tag="t1")
                        nc.gpsimd.tensor_mul(t1[:], raw[:],
                                             sb_cos[:, s0:s0 + CHUNK])
                        t2 = t2p.tile([TILE, CHUNK], BF16, tag="t2")
                        nc.vector.tensor_mul(t2[:], rot[:],
                                             sb_sin[:, s0:s0 + CHUNK])
                        dst = sb_q if m < 2 else sb_k
                        h = m % 2
                        nc.vector.tensor_add(dst[:, h, s0:s0 + CHUNK],
                                             t1[:], t2[:])
                    # v: M-tiles over s (4 per chunk), N = 2 heads * 128
                    for sm in range(CHUNK // TILE):
                        vp = pp.tile([TILE, CHUNK], F32, tag="ps")
                        st = sm * TILE
                        for e in range(EK):
                            nc.tensor.matmul(
                                vp[:, :2 * TILE], xt[:, e, st:st + TILE],
                                sb_wv[:, e, :], start=(e == 0),
                                stop=(e == EK - 1))
                        vt = ci * 4 + sm
                        for h in range(HPC):
                            nc.vector.tensor_copy(
                                sb_vg[:, h, vt, 0:TILE],
                                vp[:, h * TILE:(h + 1) * TILE])

                    # ---- block-sparse attention for this chunk's tiles ----
                    for h in range(HPC):
                        _attend_chunk(nc, ci, h, sb_q, sb_k, sb_vg, sb_attn,
                                      sb_mwx, sb_id, sb_tn, sb_oh,
                                      lgp_pool, oo, trp, wtp, nrm)

                    # ---- output projection for this chunk ----
                    ob = obp.tile([TILE, EK, CHUNK], BF16, tag="ob")
                    for m in range(EK):
                        op = pp.tile([TILE, CHUNK], F32, tag="ps")
                        for h in range(HPC):
                            nc.tensor.matmul(
                                op[:], sb_wo[:, h, m * TILE:(m + 1) * TILE],
                                sb_attn[:, h, s0:s0 + CHUNK],
                                start=(h == 0), stop=(h == HPC - 1))
                        if m % 2 == 0:
                            nc.scalar.copy(out=ob[:, m, :], in_=op[:])
                        else:
                            nc.vector.tensor_copy(ob[:, m, :], op[:])
                    nc.sync.dma_start(
                        out=outT[:, s0:s0 + CHUNK].rearrange(
                            "(a p) b -> p a b", p=TILE),
                        in_=ob[:])
    return nc


def _attend_chunk(nc, ci, h, sb_q, sb_k, sb_vg, sb_attn, sb_mwx, sb_id,
                  sb_tn, sb_oh, lgp_pool, oo, trp, wtp, nrm):
    """Attention for one (head, chunk of 4 query tiles). For each past tile
    v <= 4ci+3: ONE N=512 QK matmul against the chunk's 4 query tiles, an
    additive bias matmul (one-hot selects head h's row v; quarters where
    v > t carry -4e9, i.e. masked automatically), the diagonal tri-mask
    where v is in this chunk, exp, then per-(t,v) PV with denominator
    column. Two [128, 264] psum tiles pack the 4 query-tile accumulators."""
    s0 = ci * CHUNK
    h0 = h * T
    nv = ci * 4 + 4          # tiles 0..4ci+3 participate
    q_slab = sb_q[:, h, s0:s0 + CHUNK]
    outs = [oo.tile([TILE, 264], F32, tag="oo", name=f"oo{ci}{h}{i}")
            for i in range(2)]

    PIPE = 1
    wts = [None] * nv

    def seg(tt):
        return outs[tt // 2][:, (tt % 2) * 132:(tt % 2) * 132 + 129]

    def emit_qk(v):
        lg = lgp_pool.tile([TILE, CHUNK], F32, tag="lg")
        # bias first (start=True clears the whole bank's has_written bits)
        nc.tensor.matmul(
            lg[:],
            sb_oh[h0:h0 + T, v * TILE:(v + 1) * TILE],
            sb_mwx[h0:h0 + T, ci * CHUNK:(ci + 1) * CHUNK],
            start=True, stop=False, skip_group_check=True)
        has_diag = ci * 4 <= v
        nc.tensor.matmul(
            lg[:], sb_k[:, h, v * TILE:(v + 1) * TILE], q_slab,
            start=False, stop=not has_diag, skip_group_check=True)
        if has_diag:
            tt = v - ci * 4
            nc.tensor.matmul(
                lg[:, tt * TILE:(tt + 1) * TILE], sb_id[:], sb_tn[:],
                start=False, stop=True, skip_group_check=True)
        wt = wtp.tile([TILE, CHUNK], BF16, tag="wt")
        wts[v] = wt
        nc.scalar.activation(
            out=wt[:], in_=lg[:],
            func=mybir.ActivationFunctionType.Exp, scale=SM_SCALE)

    def emit_pv(v):
        wt = wts[v]
        for tt in range(4):
            t = ci * 4 + tt
            if v > t:
                continue
            # start=True clears has_written for the WHOLE bank, so only the
            # bank's first PV (tt even, v=0) may set it; the odd segment's
            # first write relies on overwrite-where-unset semantics.
            nc.tensor.matmul(
                seg(tt), wt[:, tt * TILE:(tt + 1) * TILE],
                sb_vg[:, h, v, 0:129],
                start=(v == 0 and tt % 2 == 0), stop=(v == t),
                skip_group_check=True)

    for v in range(min(PIPE, nv)):
        emit_qk(v)
    for v in range(nv):
        if v + PIPE < nv:
            emit_qk(v + PIPE)
        emit_pv(v)

    for tt in range(4):
        t = ci * 4 + tt
        off = (tt % 2) * 132
        out_ps = outs[tt // 2]
        rc = nrm.tile([TILE, 1], F32, tag="rc")
        nc.vector.reciprocal(out=rc[:], in_=out_ps[:, off + 128:off + 129])
        an = nrm.tile([TILE, TILE], BF16, tag="an")
        nc.vector.tensor_scalar_mul(an[:], out_ps[:, off:off + TILE], rc[:])
        tr = trp.tile([TILE, TILE], BF16, tag="tr")
        nc.tensor.transpose(tr[:], an[:], sb_id[:])
        nc.vector.tensor_copy(sb_attn[:, h, t * TILE:(t + 1) * TILE], tr[:])


def _host_prep(x, wq, wk, wv, wo, rope_angles, anchor_indices):
    xT = np.ascontiguousarray(x[0].T).astype(NPBF16)
    cos = np.cos(rope_angles.astype(np.float64))
    sin = np.sin(rope_angles.astype(np.float64))
    cosT = np.ascontiguousarray(
        np.concatenate([cos, cos], axis=1).T).astype(NPBF16)
    sinT = np.ascontiguousarray(
        np.concatenate([sin, sin], axis=1).T).astype(NPBF16)
    half = D // 2
    R = np.zeros((D, D), np.float32)
    for d in range(half):
        R[d, d + half] = -1.0
        R[d + half, d] = 1.0
    rotT = np.ascontiguousarray(R.T).astype(NPBF16)
    ident = np.eye(TILE, dtype=np.float32).astype(NPBF16)
    trineg = np.where(np.arange(TILE)[:, None] > np.arange(TILE)[None, :],
                      np.float32(NEGT), np.float32(0.0)).astype(NPBF16)

    sqrtD = math.sqrt(D)
    # one-hot selector: oh[h*T + r, t*128 + c] = 1 if r == t else 0
    ohblk = np.repeat(np.eye(T, dtype=np.float32), TILE, axis=1)
    oh = np.vstack([ohblk, ohblk]).astype(NPBF16)
    in_maps = []
    for c in range(NCORES):
        heads = [c * HPC + i for i in range(HPC)]
        wqk_c = np.concatenate(
            [wq[:, hh * D:(hh + 1) * D] for hh in heads]
            + [wk[:, hh * D:(hh + 1) * D] for hh in heads], axis=1)
        wv_c = np.concatenate([wv[:, hh * D:(hh + 1) * D] for hh in heads],
                              axis=1)
        wo_c = np.concatenate([wo[hh * D:(hh + 1) * D, :] for hh in heads],
                              axis=0)
        # bias rows: partition h*T+v holds, in column block t, the value
        # B(t, v) = sqrt(D)*ln(m) (m>0) or -4e9 (m=0 / v>t) repeated 128x.
        mwx = np.zeros((2 * T, T * TILE), np.float32)
        for i, hh in enumerate(heads):
            bmat = np.full((T, T), np.float32(NEGB), np.float32)  # [t, v]
            for t in range(T):
                sel = list(anchor_indices[0, hh, t]) + [t]
                for v in range(t + 1):
                    m = sel.count(v)
                    if m:
                        bmat[t, v] = sqrtD * math.log(m)
            for v in range(T):
                mwx[i * T + v, :] = np.repeat(bmat[:, v], TILE)
        in_maps.append({
            "xT": xT, "wqk": np.ascontiguousarray(wqk_c).astype(NPBF16),
            "wv": np.ascontiguousarray(wv_c).astype(NPBF16),
            "wo": np.ascontiguousarray(wo_c).astype(NPBF16),
            "cosT": cosT, "sinT": sinT, "rotT": rotT,
            "identD": ident, "trinegD": trineg,
            "mwxD": mwx.astype(NPBF16), "ohD": oh,
        })
    return in_maps


def kernel(x, wq, wk, wv, wo, rope_angles, anchor_indices, **run_kwargs):
    _patch_tile_drain()
    nc = build_bass()
    in_maps = _host_prep(x, wq, wk, wv, wo, rope_angles, anchor_indices)
    res = run_bass_kernel_spmd(nc, in_maps, core_ids=list(range(NCORES)),
                               **run_kwargs)
    acc = np.zeros((E, S), np.float64)
    for c in range(NCORES):
        acc += res.results[c]["outT"].astype(np.float64)
    out = np.ascontiguousarray(acc.T.reshape(B, S, E)).astype(np.float32)
    kernel.last_results = res
    return out


# revision 27
# speedup vs baseline: 1.0012x; 1.0012x over previous
"""KascadeReuseAttention Trainium2 kernel (v2).

Sharding: 16 heads / 8 cores -> 2 heads per core (head/tensor parallel).
Wq/Wk/Wv column-sharded by head, Wo row-sharded; host sums the 8 partial
outputs (the row-parallel all-reduce).

Single SPMD program for all cores: per-core anchor selection enters only as
DATA. Per (head, query-tile t) we compute block attention against ALL past
key tiles v<=t; tile multiplicities m[h,t,v] (count of v among anchors+local,
0 if unselected) are folded into the LOGITS as additive biases
B = sqrt(D)*ln(m) (-4e9 when m=0), accumulated into the logits PSUM by a
tiny K=1 matmul reading a host-precomputed bias row. exp() then yields
m*exp(s*qk) (or 0) with no per-pair vector work. The causal tri-mask for the
diagonal tile is likewise one additive matmul (identity x trineg).

The denominator rides in the PV matmul: V tiles carry an appended
ones-column, so PV produces [q, D+1] with the last column = sum of weights.
Normalization is then a per-partition reciprocal + scalar multiply, and the
[q, d] -> [d, q] transpose for the output projection is a DMA transpose.
"""

import math
import sys

import numpy as np

for _p in ("/opt/trn_rl_repo",):
    if _p not in sys.path:
        sys.path.insert(0, _p)

import ml_dtypes  # noqa: E402
import concourse.bass as bass  # noqa: E402
import concourse.mybir as mybir  # noqa: E402
import concourse.tile as tile  # noqa: E402
from concourse.bass_utils import run_bass_kernel_spmd  # noqa: E402
from concourse.vector_clock import ScopedClock  # noqa: E402

BF16 = mybir.dt.bfloat16
F32 = mybir.dt.float32
NPBF16 = ml_dtypes.bfloat16

B, S, E, H, D, K = 1, 4096, 2048, 16, 128, 8
TILE = 128
T = S // TILE          # 32 query/key tiles
NCORES = 8
HPC = H // NCORES      # heads per core = 2
CHUNK = 512            # s-chunk for projections
NCHUNK = S // CHUNK
EK = E // TILE         # 16 contraction tiles
SM_SCALE = 1.0 / math.sqrt(D)
GRP = 4                # logits tiles per psum bank
NEGB = -4e9            # additive bias for m=0 pairs (exp -> 0)
NEGT = -1e9            # additive causal mask value

_PATCHED = False


def _patch_tile_drain():
    """This container's walrus caps per-instruction sync waits; the Tile
    kernel-tail drain carries one wait per live semaphore. Split them onto
    preceding SP nops."""
    global _PATCHED
    if _PATCHED:
        return
    _PATCHED = True

    def _drain_and_barrier(self, tick_clock, wait_clock):
        nc = self.nc
        nops = []
        nsems = len(self.sems.allocated()) if self.sems is not None else 0
        for _ in range(nsems):
            nops.append(nc.sync.nop())
        drain_inst = nc.sync.drain()
        wait_clock.add_sem_waits(
            drain_inst.ins, ScopedClock({None: tick_clock.global_clock})
        )
        si = drain_inst.ins.sync_info
        waits = list(si.on_wait or [])
        if len(waits) > 1:
            si.on_wait = waits[:1]
            for i, w in enumerate(waits[1:]):
                ni = nops[i].ins if hasattr(nops[i], "ins") else nops[i]
                nsi = ni.sync_info
                if nsi is None:
                    ni.sync_info = mybir.SyncInfo(on_wait=[w], on_update=[])
                else:
                    nsi.on_wait = [w]
        nc.all_engine_barrier()
        assert self.sems is not None
        popped = nc._tile_sem_poison_stack.pop()
        assert popped is self._sem_poison
        nc.clear_and_free_semaphores(list(self.sems.allocated().values()))
        nc.all_engine_barrier()
        _split_multi_waits(nc)

    tile.TileContext._drain_and_barrier = _drain_and_barrier


def _split_multi_waits(nc):
    """Walrus here encodes at most one sync-wait per instruction; move the
    extras onto preceding same-engine no-ops."""
    ctr = [0]
    for f in nc.m.functions:
        for bb in f.blocks:
            insts = list(bb.instructions)
            if not any(
                i.sync_info and i.sync_info.on_wait
                and len(i.sync_info.on_wait) > 1
                for i in insts
            ):
                continue
            newl = []
            for inst in insts:
                si = inst.sync_info
                if si and si.on_wait and len(si.on_wait) > 1:
                    waits = list(si.on_wait)
                    for w in waits[:-1]:
                        ctr[0] += 1
                        nop = mybir.InstNoOp(
                            name=f"WSPL-{ctr[0]}", ins=[], outs=[])
                        nop.engine = inst.engine
                        nop.sync_info = mybir.SyncInfo(
                            on_wait=[w], on_update=[])
                        newl.append(nop)
                    si.on_wait = waits[-1:]
                newl.append(inst)
            bb.instructions = newl


def build_bass():
    """Uniform per-core program. Inputs (per core, bf16 unless noted):
    xT [E, S], wqk [E, 4*128] (q_h0,q_h1,k_h0,k_h1), wv [E, 256],
    wo [256, E], cosT/sinT [128, S], rotT [128,128] (R^T for rotate_half),
    ident [128,128], trineg [128,128] (additive causal mask, -1e9 below
    diag), mwx [128, T*128] (bias rows: partition h*32+t holds, at col
    j*128+i, the value sqrt(D)*ln(m[h,t,j]) or -4e9).
    Output: outT [E, S] bf16 (partial contribution of this core's heads).
    """
    nc = bass.Bass()
    xT = nc.dram_tensor("xT", [E, S], BF16, kind="ExternalInput")
    wqk = nc.dram_tensor("wqk", [E, 4 * TILE], BF16, kind="ExternalInput")
    wv = nc.dram_tensor("wv", [E, 2 * TILE], BF16, kind="ExternalInput")
    wo = nc.dram_tensor("wo", [2 * TILE, E], BF16, kind="ExternalInput")
    cosT = nc.dram_tensor("cosT", [TILE, S], BF16, kind="ExternalInput")
    sinT = nc.dram_tensor("sinT", [TILE, S], BF16, kind="ExternalInput")
    rotT = nc.dram_tensor("rotT", [TILE, TILE], BF16, kind="ExternalInput")
    identD = nc.dram_tensor("identD", [TILE, TILE], BF16,
                            kind="ExternalInput")
    trinegD = nc.dram_tensor("trinegD", [TILE, TILE], BF16,
                             kind="ExternalInput")
    mwxD = nc.dram_tensor("mwxD", [2 * T, T * TILE], BF16,
                          kind="ExternalInput")
    ohD = nc.dram_tensor("ohD", [2 * T, T * TILE], BF16,
                         kind="ExternalInput")
    outT = nc.dram_tensor("outT", [E, S], BF16, kind="ExternalOutput")

    with tile.TileContext(nc) as tc:
        with tc.tile_pool(name="const", bufs=1) as cpool:
            sb_wqk = cpool.tile([TILE, EK, 4 * TILE], BF16)
            sb_wv = cpool.tile([TILE, EK, 2 * TILE], BF16)
            sb_wo = cpool.tile([TILE, 2, E], BF16)
            sb_cos = cpool.tile([TILE, S], BF16)
            sb_sin = cpool.tile([TILE, S], BF16)
            sb_rot = cpool.tile([TILE, TILE], BF16)
            sb_id = cpool.tile([TILE, TILE], BF16)
            sb_tn = cpool.tile([TILE, TILE], BF16)
            sb_mwx = cpool.tile([2 * T, T * TILE], BF16)
            sb_oh = cpool.tile([2 * T, T * TILE], BF16)
            # persistent per-head tensors: qT/kT [d, S]; v [kv, 132] per
            # tile with col 128 = 1.0 (denominator column); attnT [d, S]
            sb_q = cpool.tile([TILE, HPC, S], BF16, tag="q")
            sb_k = cpool.tile([TILE, HPC, S], BF16, tag="k")
            sb_vg = cpool.tile([TILE, HPC, T, 132], BF16, tag="vg")
            sb_attn = cpool.tile([TILE, HPC, S], BF16, tag="attn")

            nc.sync.dma_start(out=sb_wqk[:],
                              in_=wqk.rearrange("(a p) b -> p a b", p=TILE))
            nc.sync.dma_start(out=sb_wv[:],
                              in_=wv.rearrange("(a p) b -> p a b", p=TILE))
            nc.sync.dma_start(out=sb_wo[:],
                              in_=wo.rearrange("(a p) b -> p a b", p=TILE))
            nc.sync.dma_start(out=sb_cos[:], in_=cosT[:])
            nc.sync.dma_start(out=sb_sin[:], in_=sinT[:])
            nc.sync.dma_start(out=sb_rot[:], in_=rotT[:])
            nc.sync.dma_start(out=sb_id[:], in_=identD[:])
            nc.sync.dma_start(out=sb_tn[:], in_=trinegD[:])
            nc.sync.dma_start(out=sb_mwx[:], in_=mwxD[:])
            nc.sync.dma_start(out=sb_oh[:], in_=ohD[:])
            nc.vector.memset(sb_vg[:, :, :, 128:129], 1.0)

            with (
                tc.tile_pool(name="xin", bufs=2) as xpool,
                tc.tile_pool(name="rawp", bufs=3) as rawp,
                tc.tile_pool(name="t1p", bufs=3) as t1p,
                tc.tile_pool(name="t2p", bufs=3) as t2p,
                tc.tile_pool(name="wtp", bufs=3) as wtp,
                tc.tile_pool(name="nrm", bufs=3) as nrm,
                tc.tile_pool(name="obp", bufs=2) as obp,
                tc.tile_pool(name="pp", bufs=2, space="PSUM") as pp,
                tc.tile_pool(name="lg", bufs=3, space="PSUM") as lgp_pool,
                tc.tile_pool(name="oo", bufs=2, space="PSUM") as oo,
                tc.tile_pool(name="trp", bufs=1, space="PSUM") as trp,
            ):
                for ci in range(NCHUNK):
                    s0 = ci * CHUNK
                    xt = xpool.tile([TILE, EK, CHUNK], BF16, tag="xt")
                    nc.sync.dma_start(
                        out=xt[:],
                        in_=xT[:, s0:s0 + CHUNK].rearrange(
                            "(a p) b -> p a b", p=TILE),
                    )
                    # ---- projections + RoPE for this chunk ----
                    # qT/kT M-tiles: 0=q_h0 1=q_h1 2=k_h0 3=k_h1
                    for m in range(4):
                        ps = pp.tile([TILE, CHUNK], F32, tag="ps")
                        for e in range(EK):
                            nc.tensor.matmul(
                                ps[:], sb_wqk[:, e, m * TILE:(m + 1) * TILE],
                                xt[:, e, :], start=(e == 0), stop=(e == EK - 1))
                        raw = rawp.tile([TILE, CHUNK], BF16, tag="raw")
                        nc.scalar.copy(out=raw[:], in_=ps[:])
                        rot = pp.tile([TILE, CHUNK], F32, tag="ps")
                        nc.tensor.matmul(rot[:], sb_rot[:], raw[:],
                                         start=True, stop=True)
                        t1 = t1p.tile([TILE, CHUNK], BF16, tag="t1")
                        nc.gpsimd.tensor_mul(t1[:], raw[:],
                                             sb_cos[:, s0:s0 + CHUNK])
                        t2 = t2p.tile([TILE, CHUNK], BF16, tag="t2")
                        nc.vector.tensor_mul(t2[:], rot[:],
                                             sb_sin[:, s0:s0 + CHUNK])
                        dst = sb_q if m < 2 else sb_k
                        h = m % 2
                        nc.vector.tensor_add(dst[:, h, s0:s0 + CHUNK],
                                             t1[:], t2[:])
                    # v: M-tiles over s (4 per chunk), N = 2 heads * 128
                    for sm in range(CHUNK // TILE):
                        vp = pp.tile([TILE, CHUNK], F32, tag="ps")
                        st = sm * TILE
                        for e in range(EK):
                            nc.tensor.matmul(
                                vp[:, :2 * TILE], xt[:, e, st:st + TILE],
                                sb_wv[:, e, :], start=(e == 0),
                                stop=(e == EK - 1))
                        vt = ci * 4 + sm
                        for h in range(HPC):
                            nc.vector.tensor_copy(
                                sb_vg[:, h, vt, 0:TILE],
                                vp[:, h * TILE:(h + 1) * TILE])

                    # ---- block-sparse attention for this chunk's tiles ----
                    for h in range(HPC):
                        _attend_chunk(nc, ci, h, sb_q, sb_k, sb_vg, sb_attn,
                                      sb_mwx, sb_id, sb_tn, sb_oh,
                                      lgp_pool, oo, trp, wtp, nrm)

                    # ---- output projection for this chunk ----
                    ob = obp.tile([TILE, EK, CHUNK], BF16, tag="ob")
                    for m in range(EK):
                        op = pp.tile([TILE, CHUNK], F32, tag="ps")
                        for h in range(HPC):
                            nc.tensor.matmul(
                                op[:], sb_wo[:, h, m * TILE:(m + 1) * TILE],
                                sb_attn[:, h, s0:s0 + CHUNK],
                                start=(h == 0), stop=(h == HPC - 1))
                        if m % 2 == 0:
                            nc.scalar.copy(out=ob[:, m, :], in_=op[:])
                        else:
                            nc.vector.tensor_copy(ob[:, m, :], op[:])
                    nc.sync.dma_start(
                        out=outT[:, s0:s0 + CHUNK].rearrange(
                            "(a p) b -> p a b", p=TILE),
                        in_=ob[:])
    return nc


def _attend_chunk(nc, ci, h, sb_q, sb_k, sb_vg, sb_attn, sb_mwx, sb_id,
                  sb_tn, sb_oh, lgp_pool, oo, trp, wtp, nrm):
    """Attention for one (head, chunk of 4 query tiles). For each past tile
    v <= 4ci+3: ONE N=512 QK matmul against the chunk's 4 query tiles, an
    additive bias matmul (one-hot selects head h's row v; quarters where
    v > t carry -4e9, i.e. masked automatically), the diagonal tri-mask
    where v is in this chunk, exp, then per-(t,v) PV with denominator
    column. Two [128, 264] psum tiles pack the 4 query-tile accumulators."""
    s0 = ci * CHUNK
    h0 = h * T
    nv = ci * 4 + 4          # tiles 0..4ci+3 participate
    q_slab = sb_q[:, h, s0:s0 + CHUNK]
    outs = [oo.tile([TILE, 264], F32, tag="oo", name=f"oo{ci}{h}{i}")
            for i in range(2)]

    PIPE = 2
    wts = [None] * nv

    def seg(tt):
        return outs[tt // 2][:, (tt % 2) * 132:(tt % 2) * 132 + 129]

    def emit_qk(v):
        # quarters with t < v are fully masked; trim them (they are the
        # trailing quarters only when v is inside this chunk).
        tt0 = max(0, v - ci * 4)
        nq = (4 - tt0) * TILE
        lg = lgp_pool.tile([TILE, CHUNK], F32, tag="lg")
        # bias first (start=True clears the whole bank's has_written bits)
        nc.tensor.matmul(
            lg[:, :nq],
            sb_oh[h0:h0 + T, v * TILE:(v + 1) * TILE],
            sb_mwx[h0:h0 + T, ci * CHUNK + tt0 * TILE:
                   ci * CHUNK + tt0 * TILE + nq],
            start=True, stop=False, skip_group_check=True)
        has_diag = ci * 4 <= v
        nc.tensor.matmul(
            lg[:, :nq], sb_k[:, h, v * TILE:(v + 1) * TILE],
            q_slab[:, tt0 * TILE:tt0 * TILE + nq],
            start=False, stop=not has_diag, skip_group_check=True)
        if has_diag:
            nc.tensor.matmul(
                lg[:, 0:TILE], sb_id[:], sb_tn[:],
                start=False, stop=True, skip_group_check=True)
        wt = wtp.tile([TILE, CHUNK], BF16, tag="wt")
        wts[v] = wt
        nc.scalar.activation(
            out=wt[:, :nq], in_=lg[:, :nq],
            func=mybir.ActivationFunctionType.Exp, scale=SM_SCALE)

    def emit_pv(v):
        wt = wts[v]
        tt0 = max(0, v - ci * 4)
        for tt in range(4):
            t = ci * 4 + tt
            if v > t:
                continue
            # start=True clears has_written for the WHOLE bank, so only the
            # bank's first PV (tt even, v=0) may set it; the odd segment's
            # first write relies on overwrite-where-unset semantics.
            nc.tensor.matmul(
                seg(tt), wt[:, (tt - tt0) * TILE:(tt - tt0 + 1) * TILE],
                sb_vg[:, h, v, 0:129],
                start=(v == 0 and tt % 2 == 0), stop=(v == t),
                skip_group_check=True)

    for v in range(min(PIPE, nv)):
        emit_qk(v)
    for v in range(nv):
        if v + PIPE < nv:
            emit_qk(v + PIPE)
        emit_pv(v)

    for tt in range(4):
        t = ci * 4 + tt
        off = (tt % 2) * 132
        out_ps = outs[tt // 2]
        rc = nrm.tile([TILE, 1], F32, tag="rc")
        nc.vector.reciprocal(out=rc[:], in_=out_ps[:, off + 128:off + 129])
        an = nrm.tile([TILE, TILE], BF16, tag="an")
        nc.vector.tensor_scalar_mul(an[:], out_ps[:, off:off + TILE], rc[:])
        tr = trp.tile([TILE, TILE], BF16, tag="tr")
        nc.tensor.transpose(tr[:], an[:], sb_id[:])
        nc.vector.tensor_copy(sb_attn[:, h, t * TILE:(t + 1) * TILE], tr[:])


def _host_prep(x, wq, wk, wv, wo, rope_angles, anchor_indices):
    xT = np.ascontiguousarray(x[0].T).astype(NPBF16)
    cos = np.cos(rope_angles.astype(np.float64))
    sin = np.sin(rope_angles.astype(np.float64))
    cosT = np.ascontiguousarray(
        np.concatenate([cos, cos], axis=1).T).astype(NPBF16)
    sinT = np.ascontiguousarray(
        np.concatenate([sin, sin], axis=1).T).astype(NPBF16)
    half = D // 2
    R = np.zeros((D, D), np.float32)
    for d in range(half):
        R[d, d + half] = -1.0
        R[d + half, d] = 1.0
    rotT = np.ascontiguousarray(R.T).astype(NPBF16)
    ident = np.eye(TILE, dtype=np.float32).astype(NPBF16)
    trineg = np.where(np.arange(TILE)[:, None] > np.arange(TILE)[None, :],
                      np.float32(NEGT), np.float32(0.0)).astype(NPBF16)

    sqrtD = math.sqrt(D)
    # one-hot selector: oh[h*T + r, t*128 + c] = 1 if r == t else 0
    ohblk = np.repeat(np.eye(T, dtype=np.float32), TILE, axis=1)
    oh = np.vstack([ohblk, ohblk]).astype(NPBF16)
    in_maps = []
    for c in range(NCORES):
        heads = [c * HPC + i for i in range(HPC)]
        wqk_c = np.concatenate(
            [wq[:, hh * D:(hh + 1) * D] for hh in heads]
            + [wk[:, hh * D:(hh + 1) * D] for hh in heads], axis=1)
        wv_c = np.concatenate([wv[:, hh * D:(hh + 1) * D] for hh in heads],
                              axis=1)
        wo_c = np.concatenate([wo[hh * D:(hh + 1) * D, :] for hh in heads],
                              axis=0)
        # bias rows: partition h*T+v holds, in column block t, the value
        # B(t, v) = sqrt(D)*ln(m) (m>0) or -4e9 (m=0 / v>t) repeated 128x.
        mwx = np.zeros((2 * T, T * TILE), np.float32)
        for i, hh in enumerate(heads):
            bmat = np.full((T, T), np.float32(NEGB), np.float32)  # [t, v]
            for t in range(T):
                sel = list(anchor_indices[0, hh, t]) + [t]
                for v in range(t + 1):
                    m = sel.count(v)
                    if m:
                        bmat[t, v] = sqrtD * math.log(m)
            for v in range(T):
                mwx[i * T + v, :] = np.repeat(bmat[:, v], TILE)
        in_maps.append({
            "xT": xT, "wqk": np.ascontiguousarray(wqk_c).astype(NPBF16),
            "wv": np.ascontiguousarray(wv_c).astype(NPBF16),
            "wo": np.ascontiguousarray(wo_c).astype(NPBF16),
            "cosT": cosT, "sinT": sinT, "rotT": rotT,
            "identD": ident, "trinegD": trineg,
            "mwxD": mwx.astype(NPBF16), "ohD": oh,
        })
    return in_maps


def kernel(x, wq, wk, wv, wo, rope_angles, anchor_indices, **run_kwargs):
    _patch_tile_drain()
    nc = build_bass()
    in_maps = _host_prep(x, wq, wk, wv, wo, rope_angles, anchor_indices)
    res = run_bass_kernel_spmd(nc, in_maps, core_ids=list(range(NCORES)),
                               **run_kwargs)
    acc = np.zeros((E, S), np.float64)
    for c in range(NCORES):
        acc += res.results[c]["outT"].astype(np.float64)
    out = np.ascontiguousarray(acc.T.reshape(B, S, E)).astype(np.float32)
    kernel.last_results = res
    return out


# revision 31
# speedup vs baseline: 1.0284x; 1.0271x over previous
"""KascadeReuseAttention Trainium2 kernel (v2).

Sharding: 16 heads / 8 cores -> 2 heads per core (head/tensor parallel).
Wq/Wk/Wv column-sharded by head, Wo row-sharded; host sums the 8 partial
outputs (the row-parallel all-reduce).

Single SPMD program for all cores: per-core anchor selection enters only as
DATA. Per (head, query-tile t) we compute block attention against ALL past
key tiles v<=t; tile multiplicities m[h,t,v] (count of v among anchors+local,
0 if unselected) are folded into the LOGITS as additive biases
B = sqrt(D)*ln(m) (-4e9 when m=0), accumulated into the logits PSUM by a
tiny K=1 matmul reading a host-precomputed bias row. exp() then yields
m*exp(s*qk) (or 0) with no per-pair vector work. The causal tri-mask for the
diagonal tile is likewise one additive matmul (identity x trineg).

The denominator rides in the PV matmul: V tiles carry an appended
ones-column, so PV produces [q, D+1] with the last column = sum of weights.
Normalization is then a per-partition reciprocal + scalar multiply, and the
[q, d] -> [d, q] transpose for the output projection is a DMA transpose.
"""

import math
import sys

import numpy as np

for _p in ("/opt/trn_rl_repo",):
    if _p not in sys.path:
        sys.path.insert(0, _p)

import ml_dtypes  # noqa: E402
import concourse.bass as bass  # noqa: E402
import concourse.mybir as mybir  # noqa: E402
import concourse.tile as tile  # noqa: E402
from concourse.bass_utils import run_bass_kernel_spmd  # noqa: E402
from concourse.vector_clock import ScopedClock  # noqa: E402

BF16 = mybir.dt.bfloat16
F32 = mybir.dt.float32
NPBF16 = ml_dtypes.bfloat16

B, S, E, H, D, K = 1, 4096, 2048, 16, 128, 8
TILE = 128
T = S // TILE          # 32 query/key tiles
NCORES = 8
HPC = H // NCORES      # heads per core = 2
CHUNK = 512            # s-chunk for projections
NCHUNK = S // CHUNK
EK = E // TILE         # 16 contraction tiles
SM_SCALE = 1.0 / math.sqrt(D)
GRP = 4                # logits tiles per psum bank
NEGB = -4e9            # additive bias for m=0 pairs (exp -> 0)
NEGT = -1e9            # additive causal mask value

_PATCHED = False


def _patch_tile_drain():
    """This container's walrus caps per-instruction sync waits; the Tile
    kernel-tail drain carries one wait per live semaphore. Split them onto
    preceding SP nops."""
    global _PATCHED
    if _PATCHED:
        return
    _PATCHED = True

    def _drain_and_barrier(self, tick_clock, wait_clock):
        nc = self.nc
        nops = []
        nsems = len(self.sems.allocated()) if self.sems is not None else 0
        for _ in range(nsems):
            nops.append(nc.sync.nop())
        drain_inst = nc.sync.drain()
        wait_clock.add_sem_waits(
            drain_inst.ins, ScopedClock({None: tick_clock.global_clock})
        )
        si = drain_inst.ins.sync_info
        waits = list(si.on_wait or [])
        if len(waits) > 1:
            si.on_wait = waits[:1]
            for i, w in enumerate(waits[1:]):
                ni = nops[i].ins if hasattr(nops[i], "ins") else nops[i]
                nsi = ni.sync_info
                if nsi is None:
                    ni.sync_info = mybir.SyncInfo(on_wait=[w], on_update=[])
                else:
                    nsi.on_wait = [w]
        nc.all_engine_barrier()
        assert self.sems is not None
        popped = nc._tile_sem_poison_stack.pop()
        assert popped is self._sem_poison
        nc.clear_and_free_semaphores(list(self.sems.allocated().values()))
        nc.all_engine_barrier()
        _split_multi_waits(nc)

    tile.TileContext._drain_and_barrier = _drain_and_barrier


def _split_multi_waits(nc):
    """Walrus here encodes at most one sync-wait per instruction; move the
    extras onto preceding same-engine no-ops."""
    ctr = [0]
    for f in nc.m.functions:
        for bb in f.blocks:
            insts = list(bb.instructions)
            if not any(
                i.sync_info and i.sync_info.on_wait
                and len(i.sync_info.on_wait) > 1
                for i in insts
            ):
                continue
            newl = []
            for inst in insts:
                si = inst.sync_info
                if si and si.on_wait and len(si.on_wait) > 1:
                    waits = list(si.on_wait)
                    for w in waits[:-1]:
                        ctr[0] += 1
                        nop = mybir.InstNoOp(
                            name=f"WSPL-{ctr[0]}", ins=[], outs=[])
                        nop.engine = inst.engine
                        nop.sync_info = mybir.SyncInfo(
                            on_wait=[w], on_update=[])
                        newl.append(nop)
                    si.on_wait = waits[-1:]
                newl.append(inst)
            bb.instructions = newl


def build_bass():
    """Uniform per-core program. Inputs (per core, bf16 unless noted):
    xT [E, S], wqk [E, 4*128] (q_h0,q_h1,k_h0,k_h1), wv [E, 256],
    wo [256, E], cosT/sinT [128, S], rotT [128,128] (R^T for rotate_half),
    ident [128,128], trineg [128,128] (additive causal mask, -1e9 below
    diag), mwx [128, T*128] (bias rows: partition h*32+t holds, at col
    j*128+i, the value sqrt(D)*ln(m[h,t,j]) or -4e9).
    Output: outT [E, S] bf16 (partial contribution of this core's heads).
    """
    nc = bass.Bass()
    xT = nc.dram_tensor("xT", [E, S], BF16, kind="ExternalInput")
    wqk = nc.dram_tensor("wqk", [E, 4 * TILE], BF16, kind="ExternalInput")
    wv = nc.dram_tensor("wv", [E, 2 * TILE], BF16, kind="ExternalInput")
    wo = nc.dram_tensor("wo", [2 * TILE, E], BF16, kind="ExternalInput")
    cosT = nc.dram_tensor("cosT", [TILE, S], BF16, kind="ExternalInput")
    sinT = nc.dram_tensor("sinT", [TILE, S], BF16, kind="ExternalInput")
    rotT = nc.dram_tensor("rotT", [TILE, TILE], BF16, kind="ExternalInput")
    identD = nc.dram_tensor("identD", [TILE, TILE], BF16,
                            kind="ExternalInput")
    trinegD = nc.dram_tensor("trinegD", [TILE, TILE], BF16,
                             kind="ExternalInput")
    mwxD = nc.dram_tensor("mwxD", [2 * T, T * TILE], BF16,
                          kind="ExternalInput")
    ohD = nc.dram_tensor("ohD", [2 * T, T * TILE], BF16,
                         kind="ExternalInput")
    outT = nc.dram_tensor("outT", [E, S], BF16, kind="ExternalOutput")

    with tile.TileContext(nc) as tc:
        with tc.tile_pool(name="const", bufs=1) as cpool:
            sb_wqk = cpool.tile([TILE, EK, 4 * TILE], BF16)
            sb_wv = cpool.tile([TILE, EK, 2 * TILE], BF16)
            sb_wo = cpool.tile([TILE, 2, E], BF16)
            sb_cos = cpool.tile([TILE, S], BF16)
            sb_sin = cpool.tile([TILE, S], BF16)
            sb_rot = cpool.tile([TILE, TILE], BF16)
            sb_id = cpool.tile([TILE, TILE], BF16)
            sb_tn = cpool.tile([TILE, TILE], BF16)
            sb_mwx = cpool.tile([2 * T, T * TILE], BF16)
            sb_oh = cpool.tile([2 * T, T * TILE], BF16)
            # persistent per-head tensors: qT/kT [d, S]; v [kv, 132] per
            # tile with col 128 = 1.0 (denominator column); attnT [d, S]
            sb_q = cpool.tile([TILE, HPC, S], BF16, tag="q")
            sb_k = cpool.tile([TILE, HPC, S], BF16, tag="k")
            sb_vg = cpool.tile([TILE, HPC, T, 132], BF16, tag="vg")
            sb_attn = cpool.tile([TILE, HPC, S], BF16, tag="attn")

            # startup-critical loads only; the rest are issued inside the
            # first chunk so the first projection matmuls start sooner.
            nc.sync.dma_start(out=sb_wqk[:],
                              in_=wqk.rearrange("(a p) b -> p a b", p=TILE))
            nc.sync.dma_start(out=sb_rot[:], in_=rotT[:])
            nc.vector.memset(sb_vg[:, :, :, 128:129], 1.0)

            with (
                tc.tile_pool(name="xin", bufs=2) as xpool,
                tc.tile_pool(name="rawp", bufs=3) as rawp,
                tc.tile_pool(name="t1p", bufs=3) as t1p,
                tc.tile_pool(name="t2p", bufs=3) as t2p,
                tc.tile_pool(name="wtp", bufs=3) as wtp,
                tc.tile_pool(name="nrm", bufs=3) as nrm,
                tc.tile_pool(name="obp", bufs=2) as obp,
                tc.tile_pool(name="pp", bufs=2, space="PSUM") as pp,
                tc.tile_pool(name="lg", bufs=3, space="PSUM") as lgp_pool,
                tc.tile_pool(name="oo", bufs=3, space="PSUM") as oo,
            ):
                trp = pp  # transposes ride the (attention-idle) proj pool
                for ci in range(NCHUNK):
                    s0 = ci * CHUNK
                    xt = xpool.tile([TILE, EK, CHUNK], BF16, tag="xt")
                    nc.sync.dma_start(
                        out=xt[:],
                        in_=xT[:, s0:s0 + CHUNK].rearrange(
                            "(a p) b -> p a b", p=TILE),
                    )
                    if ci == 0:
                        # deferred loads, in the order compute needs them
                        nc.sync.dma_start(out=sb_cos[:], in_=cosT[:])
                        nc.sync.dma_start(out=sb_sin[:], in_=sinT[:])
                        nc.sync.dma_start(
                            out=sb_wv[:],
                            in_=wv.rearrange("(a p) b -> p a b", p=TILE))
                        nc.sync.dma_start(out=sb_id[:], in_=identD[:])
                        nc.sync.dma_start(out=sb_tn[:], in_=trinegD[:])
                        nc.sync.dma_start(out=sb_mwx[:], in_=mwxD[:])
                        nc.sync.dma_start(out=sb_oh[:], in_=ohD[:])
                        nc.sync.dma_start(
                            out=sb_wo[:],
                            in_=wo.rearrange("(a p) b -> p a b", p=TILE))
                    # ---- projections + RoPE for this chunk ----
                    # qT/kT M-tiles: 0=q_h0 1=q_h1 2=k_h0 3=k_h1
                    for m in range(4):
                        ps = pp.tile([TILE, CHUNK], F32, tag="ps")
                        for e in range(EK):
                            nc.tensor.matmul(
                                ps[:], sb_wqk[:, e, m * TILE:(m + 1) * TILE],
                                xt[:, e, :], start=(e == 0), stop=(e == EK - 1))
                        raw = rawp.tile([TILE, CHUNK], BF16, tag="raw")
                        nc.scalar.copy(out=raw[:], in_=ps[:])
                        rot = pp.tile([TILE, CHUNK], F32, tag="ps")
                        nc.tensor.matmul(rot[:], sb_rot[:], raw[:],
                                         start=True, stop=True)
                        t1 = t1p.tile([TILE, CHUNK], BF16, tag="t1")
                        nc.gpsimd.tensor_mul(t1[:], raw[:],
                                             sb_cos[:, s0:s0 + CHUNK])
                        t2 = t2p.tile([TILE, CHUNK], BF16, tag="t2")
                        nc.vector.tensor_mul(t2[:], rot[:],
                                             sb_sin[:, s0:s0 + CHUNK])
                        dst = sb_q if m < 2 else sb_k
                        h = m % 2
                        nc.vector.tensor_add(dst[:, h, s0:s0 + CHUNK],
                                             t1[:], t2[:])
                    # v: M-tiles over s (4 per chunk), N = 2 heads * 128
                    for sm in range(CHUNK // TILE):
                        vp = pp.tile([TILE, CHUNK], F32, tag="ps")
                        st = sm * TILE
                        for e in range(EK):
                            nc.tensor.matmul(
                                vp[:, :2 * TILE], xt[:, e, st:st + TILE],
                                sb_wv[:, e, :], start=(e == 0),
                                stop=(e == EK - 1))
                        vt = ci * 4 + sm
                        for h in range(HPC):
                            nc.vector.tensor_copy(
                                sb_vg[:, h, vt, 0:TILE],
                                vp[:, h * TILE:(h + 1) * TILE])

                    # ---- block-sparse attention for this chunk's tiles ----
                    for h in range(HPC):
                        _attend_chunk(nc, ci, h, sb_q, sb_k, sb_vg, sb_attn,
                                      sb_mwx, sb_id, sb_tn, sb_oh,
                                      lgp_pool, oo, trp, wtp, nrm)

                    # ---- output projection for this chunk ----
                    ob = obp.tile([TILE, EK, CHUNK], BF16, tag="ob")
                    for m in range(EK):
                        op = pp.tile([TILE, CHUNK], F32, tag="ps")
                        for h in range(HPC):
                            nc.tensor.matmul(
                                op[:], sb_wo[:, h, m * TILE:(m + 1) * TILE],
                                sb_attn[:, h, s0:s0 + CHUNK],
                                start=(h == 0), stop=(h == HPC - 1))
                        if m % 2 == 0:
                            nc.scalar.copy(out=ob[:, m, :], in_=op[:])
                        else:
                            nc.vector.tensor_copy(ob[:, m, :], op[:])
                    nc.sync.dma_start(
                        out=outT[:, s0:s0 + CHUNK].rearrange(
                            "(a p) b -> p a b", p=TILE),
                        in_=ob[:])
    return nc


def _attend_chunk(nc, ci, h, sb_q, sb_k, sb_vg, sb_attn, sb_mwx, sb_id,
                  sb_tn, sb_oh, lgp_pool, oo, trp, wtp, nrm):
    """Attention for one (head, chunk of 4 query tiles). For each past tile
    v <= 4ci+3: ONE N=512 QK matmul against the chunk's 4 query tiles, an
    additive bias matmul (one-hot selects head h's row v; quarters where
    v > t carry -4e9, i.e. masked automatically), the diagonal tri-mask
    where v is in this chunk, exp, then per-(t,v) PV with denominator
    column. Two [128, 264] psum tiles pack the 4 query-tile accumulators."""
    s0 = ci * CHUNK
    h0 = h * T
    nv = ci * 4 + 4          # tiles 0..4ci+3 participate
    q_slab = sb_q[:, h, s0:s0 + CHUNK]
    outs = [oo.tile([TILE, 264], F32, tag="oo", name=f"oo{ci}{h}{i}")
            for i in range(2)]

    PIPE = 2
    wts = [None] * nv

    def seg(tt):
        return outs[tt // 2][:, (tt % 2) * 132:(tt % 2) * 132 + 129]

    def emit_qk(v):
        # quarters with t < v are fully masked; trim them (they are the
        # trailing quarters only when v is inside this chunk).
        tt0 = max(0, v - ci * 4)
        nq = (4 - tt0) * TILE
        lg = lgp_pool.tile([TILE, CHUNK], F32, tag="lg")
        # bias first (start=True clears the whole bank's has_written bits)
        nc.tensor.matmul(
            lg[:, :nq],
            sb_oh[h0:h0 + T, v * TILE:(v + 1) * TILE],
            sb_mwx[h0:h0 + T, ci * CHUNK + tt0 * TILE:
                   ci * CHUNK + tt0 * TILE + nq],
            start=True, stop=False, skip_group_check=True)
        has_diag = ci * 4 <= v
        nc.tensor.matmul(
            lg[:, :nq], sb_k[:, h, v * TILE:(v + 1) * TILE],
            q_slab[:, tt0 * TILE:tt0 * TILE + nq],
            start=False, stop=not has_diag, skip_group_check=True)
        if has_diag:
            nc.tensor.matmul(
                lg[:, 0:TILE], sb_id[:], sb_tn[:],
                start=False, stop=True, skip_group_check=True)
        wt = wtp.tile([TILE, CHUNK], BF16, tag="wt")
        wts[v] = wt
        nc.scalar.activation(
            out=wt[:, :nq], in_=lg[:, :nq],
            func=mybir.ActivationFunctionType.Exp, scale=SM_SCALE)

    def emit_pv(v):
        wt = wts[v]
        tt0 = max(0, v - ci * 4)
        for tt in range(4):
            t = ci * 4 + tt
            if v > t:
                continue
            # start=True clears has_written for the WHOLE bank, so only the
            # bank's first PV (tt even, v=0) may set it; the odd segment's
            # first write relies on overwrite-where-unset semantics.
            nc.tensor.matmul(
                seg(tt), wt[:, (tt - tt0) * TILE:(tt - tt0 + 1) * TILE],
                sb_vg[:, h, v, 0:129],
                start=(v == 0 and tt % 2 == 0), stop=(v == t),
                skip_group_check=True)

    for v in range(min(PIPE, nv)):
        emit_qk(v)
    for v in range(nv):
        if v + PIPE < nv:
            emit_qk(v + PIPE)
        emit_pv(v)

    for tt in range(4):
        t = ci * 4 + tt
        off = (tt % 2) * 132
        out_ps = outs[tt // 2]
        rc = nrm.tile([TILE, 1], F32, tag="rc")
        nc.vector.reciprocal(out=rc[:], in_=out_ps[:, off + 128:off + 129])
        an = nrm.tile([TILE, TILE], BF16, tag="an")
        nc.vector.tensor_scalar_mul(an[:], out_ps[:, off:off + TILE], rc[:])
        tr = trp.tile([TILE, TILE], BF16, tag="ps", name=f"tr{ci}{h}{tt}")
        nc.tensor.transpose(tr[:], an[:], sb_id[:])
        nc.vector.tensor_copy(sb_attn[:, h, t * TILE:(t + 1) * TILE], tr[:])


def _host_prep(x, wq, wk, wv, wo, rope_angles, anchor_indices):
    xT = np.ascontiguousarray(x[0].T).astype(NPBF16)
    cos = np.cos(rope_angles.astype(np.float64))
    sin = np.sin(rope_angles.astype(np.float64))
    cosT = np.ascontiguousarray(
        np.concatenate([cos, cos], axis=1).T).astype(NPBF16)
    sinT = np.ascontiguousarray(
        np.concatenate([sin, sin], axis=1).T).astype(NPBF16)
    half = D // 2
    R = np.zeros((D, D), np.float32)
    for d in range(half):
        R[d, d + half] = -1.0
        R[d + half, d] = 1.0
    rotT = np.ascontiguousarray(R.T).astype(NPBF16)
    ident = np.eye(TILE, dtype=np.float32).astype(NPBF16)
    trineg = np.where(np.arange(TILE)[:, None] > np.arange(TILE)[None, :],
                      np.float32(NEGT), np.float32(0.0)).astype(NPBF16)

    sqrtD = math.sqrt(D)
    # one-hot selector: oh[h*T + r, t*128 + c] = 1 if r == t else 0
    ohblk = np.repeat(np.eye(T, dtype=np.float32), TILE, axis=1)
    oh = np.vstack([ohblk, ohblk]).astype(NPBF16)
    in_maps = []
    for c in range(NCORES):
        heads = [c * HPC + i for i in range(HPC)]
        wqk_c = np.concatenate(
            [wq[:, hh * D:(hh + 1) * D] for hh in heads]
            + [wk[:, hh * D:(hh + 1) * D] for hh in heads], axis=1)
        wv_c = np.concatenate([wv[:, hh * D:(hh + 1) * D] for hh in heads],
                              axis=1)
        wo_c = np.concatenate([wo[hh * D:(hh + 1) * D, :] for hh in heads],
                              axis=0)
        # bias rows: partition h*T+v holds, in column block t, the value
        # B(t, v) = sqrt(D)*ln(m) (m>0) or -4e9 (m=0 / v>t) repeated 128x.
        mwx = np.zeros((2 * T, T * TILE), np.float32)
        for i, hh in enumerate(heads):
            bmat = np.full((T, T), np.float32(NEGB), np.float32)  # [t, v]
            for t in range(T):
                sel = list(anchor_indices[0, hh, t]) + [t]
                for v in range(t + 1):
                    m = sel.count(v)
                    if m:
                        bmat[t, v] = sqrtD * math.log(m)
            for v in range(T):
                mwx[i * T + v, :] = np.repeat(bmat[:, v], TILE)
        in_maps.append({
            "xT": xT, "wqk": np.ascontiguousarray(wqk_c).astype(NPBF16),
            "wv": np.ascontiguousarray(wv_c).astype(NPBF16),
            "wo": np.ascontiguousarray(wo_c).astype(NPBF16),
            "cosT": cosT, "sinT": sinT, "rotT": rotT,
            "identD": ident, "trinegD": trineg,
            "mwxD": mwx.astype(NPBF16), "ohD": oh,
        })
    return in_maps


def kernel(x, wq, wk, wv, wo, rope_angles, anchor_indices, **run_kwargs):
    _patch_tile_drain()
    nc = build_bass()
    in_maps = _host_prep(x, wq, wk, wv, wo, rope_angles, anchor_indices)
    res = run_bass_kernel_spmd(nc, in_maps, core_ids=list(range(NCORES)),
                               **run_kwargs)
    acc = np.zeros((E, S), np.float64)
    for c in range(NCORES):
        acc += res.results[c]["outT"].astype(np.float64)
    out = np.ascontiguousarray(acc.T.reshape(B, S, E)).astype(np.float32)
    kernel.last_results = res
    return out
